# revision 38
# baseline (speedup 1.0000x reference)
"""Distributed ARMAConv kernel for 8 TRN2 NeuronCores (Bass/Tile).

Reference computation (N=16384 nodes, F=64 in-feats, C=32 channels,
K=2 stacks, T=2 iterations):
    for each stack k:  xbar = x
        for i in 0..1: xbar = relu(fltr @ (xbar @ w1) + x @ w2 + b)
    out = mean over stacks                                  -> [N, 32]

Strategy:
  - Row-shard fltr across 8 cores; core m holds fltr[rows_m, :] stored
    TRANSPOSED (contraction-major, two contiguous half-arrays) so every
    TensorE tile is a large contiguous DMA read.
  - fltr is stored at rest in DRAM as FP8 E3M4, pre-scaled by 2^8 on
    the host (the 2^-8 descale is folded into w1, exactly).  This cuts
    the dominant HBM stream 4x vs f32: 32 MiB per core per pass.  The
    PE consumes fp8 at bf16 speed (no DoubleRow - E4M3 would lose too
    much precision), so the kernel is TensorE-bound at ~110 us/pass.
  - Fuse the two independent ARMA stacks: Y = [xbar_k0 @ w1_k0 |
    xbar_k1 @ w1_k1] is [N, 64], so fltr streams only once per
    iteration.
  - All big matmuls run transposed (out^T = Y^T @ fltr_m^T) so fltr is
    the 512-wide moving operand (128 elem/cycle); Y tiles are the
    stationary operand (weight loads hide under the previous matmul).
  - Iteration 0 needs no communication (x is replicated).  Pass 1 runs
    in two output-row halves (full-width fltr^T streams, 1 KiB DMA
    lines - narrower strips choke the HWDGE descriptor ring); each half
    feeds TWO 512-row PSUM accumulators and fires TWO small (64 KiB)
    Y1 all-gathers, so pass 2 can consume gather chunks as they land.
    A dummy warm-up collective at t=0 absorbs the one-time rendezvous
    barrier + Mesh warm-up (~60us) that would otherwise delay gather 0.
  - Pass 2 consumes the gathered chunks contraction-major (chunk 0..2
    feed all four output accumulators, chunk 3 is processed per output
    half so the first half's epilogue hides under the second half's
    stream); chunk 3 is not needed until ~85us after pass 2 starts,
    tolerating inter-core start skew.
  - Big fltr DMAs ride the sync-engine HWDGE ring; all small/latency
    DMAs ride the scalar-engine ring so they never queue behind a
    1 MiB fltr read; collectives keep the gpsimd queue.
  - relu positive homogeneity folds the final stack-mean 0.5 scale into
    the pass-2 activation; the host only shards/quantizes inputs and
    concatenates/transposes the [32, 2048] per-core outputs.
"""

import numpy as np
import ml_dtypes

import concourse.mybir as mybir
import concourse.tile as tile
from concourse import bacc
from concourse.bass_utils import run_bass_kernel_spmd

N = 16384            # nodes
F = 64               # input features
C = 32               # channels per stack
C2 = 2 * C           # fused channels (2 stacks)
NCORES = 8
R = N // NCORES      # fltr rows per core (2048)
P = 128              # partitions
NKT = N // P         # K tiles per full pass (128)
HW_ = R // 2         # 1024 output rows per half-array
CW = 512             # output rows per pass-1 chunk / PSUM accumulator
NCH = R // CW        # 4 pass-1 chunks (each with its own all-gather)
KB1 = 4              # K tiles per pass-1 fltr DMA (512 KiB fp8 reads;
                     # 512-row tiles match pass-2's contraction blocks)
FSCALE = 256.0       # power-of-2 fp8 pre-scale (folded into w1)

F32 = mybir.dt.float32
F32R = mybir.dt.float32r
BF16 = mybir.dt.bfloat16
F8 = mybir.dt.float8e3

_CACHE = {}


def _build():
    nc = bacc.Bacc(
        trn_type="TRN2", target_bir_lowering=False, debug=False,
        num_devices=NCORES,
    )
    fltrT0_e = nc.dram_tensor("fltrt0", [N, HW_], F8, kind="ExternalInput")
    fltrT1_e = nc.dram_tensor("fltrt1", [N, HW_], F8, kind="ExternalInput")
    xT_e = nc.dram_tensor("xt", [F, N], BF16, kind="ExternalInput")
    xtm_e = nc.dram_tensor("xtm", [F, R], F32, kind="ExternalInput")
    w1i0_e = nc.dram_tensor("w1i0", [F, C2], BF16, kind="ExternalInput")
    w1i1_e = nc.dram_tensor("w1i1", [C2, C2], BF16, kind="ExternalInput")
    w2i0_e = nc.dram_tensor("w2i0", [F, C2], F32, kind="ExternalInput")
    w2i1_e = nc.dram_tensor("w2i1", [F, C2], F32, kind="ExternalInput")
    bi0_e = nc.dram_tensor("bi0", [C2, 1], F32, kind="ExternalInput")
    bi1h_e = nc.dram_tensor("bi1h", [C2, 1], F32, kind="ExternalInput")
    out_e = nc.dram_tensor("out", [C, R], F32, kind="ExternalOutput")

    RG = [list(range(NCORES))]
    fltr_halves = [fltrT0_e, fltrT1_e]

    with tile.TileContext(nc) as tc:
        with (
            tc.tile_pool(name="wpool", bufs=1) as wpool,
            tc.tile_pool(name="kpool", bufs=1) as kpool,
            tc.tile_pool(name="y0pool", bufs=1) as y0pool,
            tc.tile_pool(name="ygpool", bufs=1) as ygpool,
            tc.tile_pool(name="fpool", bufs=8) as fpool,
            tc.tile_pool(name="xbpool", bufs=2) as xbpool,
            tc.tile_pool(name="ylpool", bufs=2) as ylpool,
            tc.tile_pool(name="opool", bufs=1) as opool,
            tc.tile_pool(name="pacc", bufs=4, space="PSUM") as pacc,
            tc.tile_pool(name="psmall", bufs=2, space="PSUM") as psmall,
            tc.tile_pool(name="dram", bufs=8, space="DRAM") as dram,
        ):
            # w1i0 and the xT quarters first: they gate the first Y0 matmul
            # and thus the whole pass-1 PE start.  Four independent tiles
            # (no write-after-write chain) ride the sync ring AHEAD of the
            # fltr stream - the scalar ring's per-DMA fixed costs would
            # deliver them too slowly.
            w1i0 = wpool.tile([F, C2], BF16)
            nc.scalar.dma_start(w1i0[:], w1i0_e[:])
            xfs = []
            for g in range(4):
                xf = wpool.tile([F, N // 4], BF16, name=f"xf{g}")
                nc.sync.dma_start(xf[:],
                                  xT_e[:, g * (N // 4):(g + 1) * (N // 4)])
                xfs.append(xf)

            # remaining resident small tensors
            w1i1 = wpool.tile([C2, C2], BF16)  # block-diag [w1_k0i1, w1_k1i1]
            nc.scalar.dma_start(w1i1[:], w1i1_e[:])
            w2i0 = wpool.tile([F, C2], F32R)
            nc.scalar.dma_start(w2i0[:], w2i0_e[:].bitcast(F32R))
            bi0 = wpool.tile([C2, 1], F32)
            nc.scalar.dma_start(bi0[:], bi0_e[:])
            xm = wpool.tile([F, R], F32R)
            nc.scalar.dma_start(xm[:], xtm_e[:].bitcast(F32R))
            w2i1 = wpool.tile([F, C2], F32R)
            nc.scalar.dma_start(w2i1[:], w2i1_e[:].bitcast(F32R))
            bi1h = wpool.tile([C2, 1], F32)
            nc.scalar.dma_start(bi1h[:], bi1h_e[:])

            y0 = y0pool.tile([P, NKT, C2], BF16, tag="y0")  # node-major Y0

            # ---- Y0 = x @ [w1_k0i0 | w1_k1i0], node-major, cast to bf16 ----
            for g in range(16):  # 16 groups of 8 kt
                xf = xfs[g // 4]
                off = (g % 4) * 1024
                ps0 = psmall.tile([P, 8, C2], F32, name="ps0", tag="ps0")
                for i in range(8):
                    nc.tensor.matmul(
                        ps0[:, i, :],
                        xf[:, off + i * P:off + (i + 1) * P],
                        w1i0[:],
                        start=True, stop=True,
                    )
                nc.vector.tensor_copy(y0[:, g * 8:(g + 1) * 8, :], ps0[:])

            # pass-2 gather-half tiles: yg tile h holds gather half h; row
            # b*128+p of gout_h is node (b//8)*2048 + h*1024 + (b%8)*128 + p.
            yg = [ygpool.tile([P, NCORES * 8, C2], BF16, name=f"yg{h}",
                              tag=f"yg{h}") for h in range(2)]
            yg_issued = [False] * 2
            gouts = []

            def issue_yg(c):
                # deferred issue: by the time it is queued the gather is
                # (normally) complete, so the scalar ring never head-blocks
                if not yg_issued[c]:
                    nc.scalar.dma_start(
                        yg[c][:],
                        gouts[c][:].rearrange("(b p) ch -> p b ch", p=P),
                    )
                    yg_issued[c] = True

            def issue_yg0():
                issue_yg(0)

            # ---- pass 1: two output-row halves (full-width 1 KiB DMA
            # ---- lines); each half fills two 512-row accumulators and
            # ---- fires two small all-gathers back-to-back
            kept = {}
            for half in range(2):
                p1 = []
                for rc2 in range(2):
                    ck = half * 2 + rc2
                    acc = pacc.tile([C2, CW], F32, name=f"p1_{ck}",
                                    tag="acc")
                    nc.tensor.matmul(
                        acc[:],
                        w2i0[:],
                        xm[:, ck * CW:(ck + 1) * CW],
                        start=True, stop=False,
                    )
                    p1.append(acc)
                for ktb in range(NKT // KB1):
                    if ktb % 8 in (4, 5):
                        # contraction rows [j*2048, +1024) for odd j: pin
                        # every other phase-A block in SBUF so phase A's
                        # steady-state fltr demand is halved while the last
                        # gather's data phase competes for DMA bandwidth
                        ft = kpool.tile([P, KB1, HW_], F8, name="ftk",
                                        tag="ftk", bufs=16)
                        kept[(half, ktb // 4, ktb % 4)] = ft
                    else:
                        ft = fpool.tile([P, KB1, HW_], F8, name="ft",
                                        tag="ft")
                    nc.sync.dma_start(
                        ft[:],
                        fltr_halves[half][ktb * KB1 * P:(ktb + 1) * KB1 * P,
                                          :]
                        .rearrange("(b p) c -> p b c", p=P),
                    )
                    for b in range(KB1):
                        kt = ktb * KB1 + b
                        for rc2 in range(2):
                            nc.tensor.matmul(
                                p1[rc2][:],
                                y0[:, kt, :],
                                ft[:, b, rc2 * CW:(rc2 + 1) * CW],
                                start=False, stop=(kt == NKT - 1),
                            )

                # epilogue: relu -> Y1 half (bf16) -> one small all-gather
                y1h = ylpool.tile([P, 8, C2], BF16, name="y1h")
                for rc2 in range(2):
                    xb1 = xbpool.tile([C2, CW], BF16, name="xb1")
                    nc.scalar.activation(
                        xb1[:], p1[rc2][:],
                        mybir.ActivationFunctionType.Relu,
                        bias=bi0[:], scale=1.0,
                    )
                    for t in range(4):  # node-subtiles of 128 in the chunk
                        psy = psmall.tile([P, C2], F32, name="psy",
                                          tag="psy")
                        nc.tensor.matmul(
                            psy[:],
                            xb1[:, t * P:(t + 1) * P],
                            w1i1[:],
                            start=True, stop=True,
                        )
                        nc.vector.tensor_copy(y1h[:, rc2 * 4 + t, :],
                                              psy[:])
                gin = dram.tile([HW_, C2], BF16, name="gin", tag="gin",
                                bufs=2)
                nc.scalar.dma_start(
                    gin[:].rearrange("(t p) ch -> p t ch", p=P),
                    y1h[:],
                )
                gout = dram.tile(
                    [NCORES * HW_, C2], BF16, name="gout", tag="gout",
                    addr_space="Shared", bufs=2,
                )
                nc.gpsimd.collective_compute(
                    "AllGather", mybir.AluOpType.bypass,
                    replica_groups=RG,
                    ins=[gin[:].opt()], outs=[gout[:].opt()],
                )
                gouts.append(gout)
            # gather-0 finished well before pass-1 ends; load its yg tile
            # now so pass-2's first matmuls start without a long wait
            issue_yg(0)

            outT = opool.tile([C, R], F32)

            # ---- pass 2: contraction is gathered Y1, consumed chunk-major
            p2 = []
            for rc in range(NCH):
                acc = pacc.tile([C2, CW], F32, name=f"p2_{rc}", tag="acc")
                nc.tensor.matmul(
                    acc[:],
                    w2i1[:],
                    xm[:, rc * CW:(rc + 1) * CW],
                    start=True, stop=False,
                )
                p2.append(acc)

            def p2_block(h, j, hx, rcs, stop):
                # contraction rows [j*2048 + h*1024, +1024) of half-array hx
                for b2 in range(2):
                    if h == 0 and j % 2 == 1:
                        ft = kept[(hx, j, b2)]  # pinned in SBUF since pass 1
                    else:
                        ft = fpool.tile([P, 4, HW_], F8, name="ft2",
                                        tag="ft")
                        nc.sync.dma_start(
                            ft[:],
                            fltr_halves[hx][j * R + h * HW_ + b2 * CW:
                                            j * R + h * HW_ + (b2 + 1) * CW,
                                            :]
                            .rearrange("(b p) c -> p b c", p=P),
                        )
                    for b in range(4):
                        for rc in rcs:
                            nc.tensor.matmul(
                                p2[rc][:],
                                yg[h][:, j * 8 + b2 * 4 + b, :],
                                ft[:, b, (rc % 2) * CW:(rc % 2 + 1) * CW],
                                start=False,
                                stop=stop and b2 == 1 and b == 3
                                and rc == rcs[-1],
                            )

            # phase A: gather half 0 feeds all four output accumulators;
            # j<4 runs from pinned tiles while gather 1's data phase is
            # still in flight
            issue_yg(1)
            for j in range(NCORES):
                p2_block(0, j, 0, [0, 1], False)
                p2_block(0, j, 1, [2, 3], False)

            def p2_epilogue(rc):
                xb2 = xbpool.tile([C2, CW], F32, name="xb2")
                nc.scalar.activation(
                    xb2[:], p2[rc][:], mybir.ActivationFunctionType.Relu,
                    bias=bi1h[:], scale=0.5,
                )
                # partition-shift stack-1 half to base 0 (DMA), then add
                xs = xbpool.tile([C, CW], F32, name="xs")
                nc.scalar.dma_start(xs[:], xb2[C:C2, :])
                nc.vector.tensor_add(
                    outT[:, rc * CW:(rc + 1) * CW],
                    xb2[0:C, :], xs[:],
                )

            # phase B: gather half 1 per output half; the first half's
            # epilogue hides under the second half's matmul stream
            for j in range(NCORES):
                p2_block(1, j, 0, [0, 1], j == NCORES - 1)
            for rc in (0, 1):
                p2_epilogue(rc)
            nc.scalar.dma_start(out_e[:, 0:HW_], outT[:, 0:HW_])
            for j in range(NCORES):
                p2_block(1, j, 1, [2, 3], j == NCORES - 1)
            for rc in (2, 3):
                p2_epilogue(rc)
            nc.scalar.dma_start(out_e[:, HW_:R], outT[:, HW_:R])

    nc.compile()
    return nc


def kernel(**inputs):
    x = np.ascontiguousarray(np.asarray(inputs["x"], dtype=np.float32))
    fltr = np.ascontiguousarray(np.asarray(inputs["fltr"], dtype=np.float32))

    def cat(a, b, axis=1):
        return np.ascontiguousarray(
            np.concatenate(
                [np.asarray(a, np.float32), np.asarray(b, np.float32)],
                axis=axis,
            )
        )

    f8 = ml_dtypes.float8_e3m4
    bf = ml_dtypes.bfloat16
    w1i0 = np.ascontiguousarray(
        (cat(inputs["k0i0_w1"], inputs["k1i0_w1"]) / FSCALE).astype(bf))
    w1i1f = np.zeros((C2, C2), dtype=np.float32)
    w1i1f[0:C, 0:C] = np.asarray(inputs["k0i1_w1"], np.float32)
    w1i1f[C:C2, C:C2] = np.asarray(inputs["k1i1_w1"], np.float32)
    w1i1 = np.ascontiguousarray((w1i1f / FSCALE).astype(bf))
    w2i0 = cat(inputs["k0i0_w2"], inputs["k1i0_w2"])
    w2i1 = cat(inputs["k0i1_w2"], inputs["k1i1_w2"])
    bi0 = cat(inputs["k0i0_b"], inputs["k1i0_b"], axis=0)[:, None]
    bi1h = 0.5 * cat(inputs["k0i1_b"], inputs["k1i1_b"], axis=0)[:, None]
    bi1h = np.ascontiguousarray(bi1h)
    xT = np.ascontiguousarray(x.T.astype(bf))
    # fp8 E3M4 fltr at rest: transpose per core, scale by 2^8 (descale is
    # folded into w1i0/w1i1 above; values land in [-10.9, 10.9] < 15.5 max)
    fltrs = (fltr * np.float32(FSCALE)).astype(f8)

    if "nc" not in _CACHE:
        _CACHE["nc"] = _build()
    nc = _CACHE["nc"]

    in_maps = []
    for m in range(NCORES):
        rows = slice(m * R, (m + 1) * R)
        in_maps.append({
            "fltrt0": np.ascontiguousarray(fltrs[m * R:m * R + HW_, :].T),
            "fltrt1": np.ascontiguousarray(fltrs[m * R + HW_:(m + 1) * R, :].T),
            "xt": xT,
            "xtm": np.ascontiguousarray(x[rows, :].T),
            "w1i0": w1i0, "w1i1": w1i1, "w2i0": w2i0, "w2i1": w2i1,
            "bi0": bi0, "bi1h": bi1h,
        })

    import os
    import time
    trace = os.environ.get("ARMA_TRACE") == "1"
    last_exc = None
    for attempt in range(3):
        try:
            res = run_bass_kernel_spmd(
                nc, in_maps, core_ids=list(range(NCORES)), trace=trace,
            )
            break
        except Exception as e:  # transient NRT device errors: retry
            last_exc = e
            time.sleep(5.0)
    else:
        raise last_exc
    _CACHE["last_results"] = res
    out = np.concatenate(
        [np.asarray(res.results[m]["out"]).T for m in range(NCORES)], axis=0
    )
    return out


# revision 41
# speedup vs baseline: 1.1660x; 1.1660x over previous
"""Distributed ARMAConv kernel for 8 TRN2 NeuronCores (Bass/Tile).

Reference computation (N=16384 nodes, F=64 in-feats, C=32 channels,
K=2 stacks, T=2 iterations):
    for each stack k:  xbar = x
        for i in 0..1: xbar = relu(fltr @ (xbar @ w1) + x @ w2 + b)
    out = mean over stacks                                  -> [N, 32]

Strategy:
  - Row-shard fltr across 8 cores; core m holds fltr[rows_m, :] stored
    TRANSPOSED (contraction-major, two contiguous half-arrays) so every
    TensorE tile is a large contiguous DMA read.
  - fltr is stored at rest in DRAM as FP8 E3M4, pre-scaled by 2^8 on
    the host (the 2^-8 descale is folded into w1, exactly).  This cuts
    the dominant HBM stream 4x vs f32: 32 MiB per core per pass.  The
    PE consumes fp8 at bf16 speed (no DoubleRow - E4M3 would lose too
    much precision), so the kernel is TensorE-bound at ~110 us/pass.
  - Fuse the two independent ARMA stacks: Y = [xbar_k0 @ w1_k0 |
    xbar_k1 @ w1_k1] is [N, 64], so fltr streams only once per
    iteration.
  - All big matmuls run transposed (out^T = Y^T @ fltr_m^T) so fltr is
    the 512-wide moving operand (128 elem/cycle); Y tiles are the
    stationary operand (weight loads hide under the previous matmul).
  - Iteration 0 needs no communication (x is replicated).  Pass 1 runs
    in two output-row halves (full-width fltr^T streams, 1 KiB DMA
    lines - narrower strips choke the HWDGE descriptor ring); each half
    feeds TWO 512-row PSUM accumulators and fires TWO small (64 KiB)
    Y1 all-gathers, so pass 2 can consume gather chunks as they land.
    A dummy warm-up collective at t=0 absorbs the one-time rendezvous
    barrier + Mesh warm-up (~60us) that would otherwise delay gather 0.
  - Pass 2 consumes the gathered chunks contraction-major (chunk 0..2
    feed all four output accumulators, chunk 3 is processed per output
    half so the first half's epilogue hides under the second half's
    stream); chunk 3 is not needed until ~85us after pass 2 starts,
    tolerating inter-core start skew.
  - Big fltr DMAs ride the sync-engine HWDGE ring; all small/latency
    DMAs ride the scalar-engine ring so they never queue behind a
    1 MiB fltr read; collectives keep the gpsimd queue.
  - relu positive homogeneity folds the final stack-mean 0.5 scale into
    the pass-2 activation; the host only shards/quantizes inputs and
    concatenates/transposes the [32, 2048] per-core outputs.
"""

import numpy as np
import ml_dtypes

import concourse.mybir as mybir
import concourse.tile as tile
from concourse import bacc
from concourse.bass_utils import run_bass_kernel_spmd

N = 16384            # nodes
F = 64               # input features
C = 32               # channels per stack
C2 = 2 * C           # fused channels (2 stacks)
NCORES = 8
R = N // NCORES      # fltr rows per core (2048)
P = 128              # partitions
NKT = N // P         # K tiles per full pass (128)
HW_ = R // 2         # 1024 output rows per half-array
CW = 512             # output rows per pass-1 chunk / PSUM accumulator
NCH = R // CW        # 4 pass-1 chunks (each with its own all-gather)
KB1 = 4              # K tiles per pass-1 fltr DMA (512 KiB fp8 reads;
                     # 512-row tiles match pass-2's contraction blocks)
FSCALE = 256.0       # power-of-2 fp8 pre-scale (folded into w1)

F32 = mybir.dt.float32
F32R = mybir.dt.float32r
BF16 = mybir.dt.bfloat16
F8 = mybir.dt.float8e3

_CACHE = {}


def _build():
    nc = bacc.Bacc(
        trn_type="TRN2", target_bir_lowering=False, debug=False,
        num_devices=NCORES,
    )
    fltrT0_e = nc.dram_tensor("fltrt0", [N, HW_], F8, kind="ExternalInput")
    fltrT1_e = nc.dram_tensor("fltrt1", [N, HW_], F8, kind="ExternalInput")
    xT_e = nc.dram_tensor("xt", [F, N], BF16, kind="ExternalInput")
    xtm_e = nc.dram_tensor("xtm", [F, R], F32, kind="ExternalInput")
    w1i0_e = nc.dram_tensor("w1i0", [F, C2], BF16, kind="ExternalInput")
    w1i1_e = nc.dram_tensor("w1i1", [C2, C2], BF16, kind="ExternalInput")
    w2i0_e = nc.dram_tensor("w2i0", [F, C2], F32, kind="ExternalInput")
    w2i1_e = nc.dram_tensor("w2i1", [F, C2], F32, kind="ExternalInput")
    bi0_e = nc.dram_tensor("bi0", [C2, 1], F32, kind="ExternalInput")
    bi1h_e = nc.dram_tensor("bi1h", [C2, 1], F32, kind="ExternalInput")
    out_e = nc.dram_tensor("out", [C, R], F32, kind="ExternalOutput")

    RG = [list(range(NCORES))]
    fltr_halves = [fltrT0_e, fltrT1_e]

    with tile.TileContext(nc) as tc:
        with (
            tc.tile_pool(name="wpool", bufs=1) as wpool,
            tc.tile_pool(name="kpool", bufs=1) as kpool,
            tc.tile_pool(name="y0pool", bufs=1) as y0pool,
            tc.tile_pool(name="ygpool", bufs=1) as ygpool,
            tc.tile_pool(name="fpool", bufs=8) as fpool,
            tc.tile_pool(name="xbpool", bufs=2) as xbpool,
            tc.tile_pool(name="ylpool", bufs=2) as ylpool,
            tc.tile_pool(name="opool", bufs=1) as opool,
            tc.tile_pool(name="pacc", bufs=4, space="PSUM") as pacc,
            tc.tile_pool(name="psmall", bufs=2, space="PSUM") as psmall,
            tc.tile_pool(name="dram", bufs=8, space="DRAM") as dram,
        ):
            # w1i0 and the xT quarters first: they gate the first Y0 matmul
            # and thus the whole pass-1 PE start.  Four independent tiles
            # (no write-after-write chain) ride the sync ring AHEAD of the
            # fltr stream - the scalar ring's per-DMA fixed costs would
            # deliver them too slowly.
            w1i0 = wpool.tile([F, C2], BF16)
            nc.scalar.dma_start(w1i0[:], w1i0_e[:])
            xfs = []
            for g in range(4):
                xf = wpool.tile([F, N // 4], BF16, name=f"xf{g}")
                nc.sync.dma_start(xf[:],
                                  xT_e[:, g * (N // 4):(g + 1) * (N // 4)])
                xfs.append(xf)

            # dummy warm-up collective: anchors the one-time rendezvous
            # barrier at t~20us while all cores are still in startup (the
            # barrier attaches to each core's FIRST collective; without
            # this it attaches to gather-0 mid-pass and costs ~80us).
            # Collectives cannot read IO tensors: bounce w1i0 via DRAM.
            gwin = dram.tile([F, C2], BF16, name="gwin", tag="gwin")
            nc.scalar.dma_start(gwin[:], w1i0[:])
            gwout = dram.tile([NCORES * F, C2], BF16, name="gwout",
                              tag="gwout", addr_space="Shared")
            nc.gpsimd.collective_compute(
                "AllGather", mybir.AluOpType.bypass,
                replica_groups=RG,
                ins=[gwin[:].opt()], outs=[gwout[:].opt()],
            )

            # remaining resident small tensors
            w1i1 = wpool.tile([C2, C2], BF16)  # block-diag [w1_k0i1, w1_k1i1]
            nc.scalar.dma_start(w1i1[:], w1i1_e[:])
            w2i0 = wpool.tile([F, C2], F32R)
            nc.scalar.dma_start(w2i0[:], w2i0_e[:].bitcast(F32R))
            bi0 = wpool.tile([C2, 1], F32)
            nc.scalar.dma_start(bi0[:], bi0_e[:])
            xm = wpool.tile([F, R], F32R)
            nc.scalar.dma_start(xm[:], xtm_e[:].bitcast(F32R))
            w2i1 = wpool.tile([F, C2], F32R)
            nc.scalar.dma_start(w2i1[:], w2i1_e[:].bitcast(F32R))
            bi1h = wpool.tile([C2, 1], F32)
            nc.scalar.dma_start(bi1h[:], bi1h_e[:])

            y0 = y0pool.tile([P, NKT, C2], BF16, tag="y0")  # node-major Y0

            # ---- Y0 = x @ [w1_k0i0 | w1_k1i0], node-major, cast to bf16 ----
            for g in range(16):  # 16 groups of 8 kt
                xf = xfs[g // 4]
                off = (g % 4) * 1024
                ps0 = psmall.tile([P, 8, C2], F32, name="ps0", tag="ps0")
                for i in range(8):
                    nc.tensor.matmul(
                        ps0[:, i, :],
                        xf[:, off + i * P:off + (i + 1) * P],
                        w1i0[:],
                        start=True, stop=True,
                    )
                nc.vector.tensor_copy(y0[:, g * 8:(g + 1) * 8, :], ps0[:])

            # pass-2 gather-half tiles: yg tile h holds gather half h; row
            # b*128+p of gout_h is node (b//8)*2048 + h*1024 + (b%8)*128 + p.
            yg = [ygpool.tile([P, NCORES * 8, C2], BF16, name=f"yg{h}",
                              tag=f"yg{h}") for h in range(2)]
            yg_issued = [False] * 2
            gouts = []

            def issue_yg(c):
                # deferred issue: by the time it is queued the gather is
                # (normally) complete, so the scalar ring never head-blocks
                if not yg_issued[c]:
                    nc.scalar.dma_start(
                        yg[c][:],
                        gouts[c][:].rearrange("(b p) ch -> p b ch", p=P),
                    )
                    yg_issued[c] = True

            def issue_yg0():
                issue_yg(0)

            # ---- pass 1: two output-row halves (full-width 1 KiB DMA
            # ---- lines); each half fills two 512-row accumulators and
            # ---- fires two small all-gathers back-to-back
            kept = {}
            for half in range(2):
                p1 = []
                for rc2 in range(2):
                    ck = half * 2 + rc2
                    acc = pacc.tile([C2, CW], F32, name=f"p1_{ck}",
                                    tag="acc")
                    nc.tensor.matmul(
                        acc[:],
                        w2i0[:],
                        xm[:, ck * CW:(ck + 1) * CW],
                        start=True, stop=False,
                    )
                    p1.append(acc)
                for ktb in range(NKT // KB1):
                    if ktb < 16 and ktb % 4 < 2:
                        # contraction rows [j*2048, +1024) for j<4: the
                        # blocks pass-2 phase A consumes FIRST - pin them
                        # in SBUF so phase A's opening runs DMA-free while
                        # the last gather's data phase is still in flight
                        ft = kpool.tile([P, KB1, HW_], F8, name="ftk",
                                        tag="ftk", bufs=16)
                        kept[(half, ktb // 4, ktb % 4)] = ft
                    else:
                        ft = fpool.tile([P, KB1, HW_], F8, name="ft",
                                        tag="ft")
                    nc.sync.dma_start(
                        ft[:],
                        fltr_halves[half][ktb * KB1 * P:(ktb + 1) * KB1 * P,
                                          :]
                        .rearrange("(b p) c -> p b c", p=P),
                    )
                    for b in range(KB1):
                        kt = ktb * KB1 + b
                        for rc2 in range(2):
                            nc.tensor.matmul(
                                p1[rc2][:],
                                y0[:, kt, :],
                                ft[:, b, rc2 * CW:(rc2 + 1) * CW],
                                start=False, stop=(kt == NKT - 1),
                            )

                # epilogue: relu -> Y1 half (bf16) -> one small all-gather
                y1h = ylpool.tile([P, 8, C2], BF16, name="y1h")
                for rc2 in range(2):
                    xb1 = xbpool.tile([C2, CW], BF16, name="xb1")
                    nc.scalar.activation(
                        xb1[:], p1[rc2][:],
                        mybir.ActivationFunctionType.Relu,
                        bias=bi0[:], scale=1.0,
                    )
                    for t in range(4):  # node-subtiles of 128 in the chunk
                        psy = psmall.tile([P, C2], F32, name="psy",
                                          tag="psy")
                        nc.tensor.matmul(
                            psy[:],
                            xb1[:, t * P:(t + 1) * P],
                            w1i1[:],
                            start=True, stop=True,
                        )
                        nc.vector.tensor_copy(y1h[:, rc2 * 4 + t, :],
                                              psy[:])
                gin = dram.tile([HW_, C2], BF16, name="gin", tag="gin",
                                bufs=2)
                nc.scalar.dma_start(
                    gin[:].rearrange("(t p) ch -> p t ch", p=P),
                    y1h[:],
                )
                gout = dram.tile(
                    [NCORES * HW_, C2], BF16, name="gout", tag="gout",
                    addr_space="Shared", bufs=2,
                )
                nc.gpsimd.collective_compute(
                    "AllGather", mybir.AluOpType.bypass,
                    replica_groups=RG,
                    ins=[gin[:].opt()], outs=[gout[:].opt()],
                )
                gouts.append(gout)
            # gather-0 finished well before pass-1 ends; load its yg tile
            # now so pass-2's first matmuls start without a long wait
            issue_yg(0)

            outT = opool.tile([C, R], F32)

            # ---- pass 2: contraction is gathered Y1, consumed chunk-major
            p2 = []
            for rc in range(NCH):
                acc = pacc.tile([C2, CW], F32, name=f"p2_{rc}", tag="acc")
                nc.tensor.matmul(
                    acc[:],
                    w2i1[:],
                    xm[:, rc * CW:(rc + 1) * CW],
                    start=True, stop=False,
                )
                p2.append(acc)

            def p2_block(h, j, hx, rcs, stop):
                # contraction rows [j*2048 + h*1024, +1024) of half-array hx
                for b2 in range(2):
                    if h == 0 and j < 4:
                        ft = kept[(hx, j, b2)]  # pinned in SBUF since pass 1
                    else:
                        ft = fpool.tile([P, 4, HW_], F8, name="ft2",
                                        tag="ft")
                        nc.sync.dma_start(
                            ft[:],
                            fltr_halves[hx][j * R + h * HW_ + b2 * CW:
                                            j * R + h * HW_ + (b2 + 1) * CW,
                                            :]
                            .rearrange("(b p) c -> p b c", p=P),
                        )
                    for b in range(4):
                        for rc in rcs:
                            nc.tensor.matmul(
                                p2[rc][:],
                                yg[h][:, j * 8 + b2 * 4 + b, :],
                                ft[:, b, (rc % 2) * CW:(rc % 2 + 1) * CW],
                                start=False,
                                stop=stop and b2 == 1 and b == 3
                                and rc == rcs[-1],
                            )

            # phase A: gather half 0 feeds all four output accumulators;
            # j<4 runs from pinned tiles while gather 1's data phase is
            # still in flight
            issue_yg(1)
            for j in range(NCORES):
                p2_block(0, j, 0, [0, 1], False)
                p2_block(0, j, 1, [2, 3], False)

            def p2_epilogue(rc):
                xb2 = xbpool.tile([C2, CW], F32, name="xb2")
                nc.scalar.activation(
                    xb2[:], p2[rc][:], mybir.ActivationFunctionType.Relu,
                    bias=bi1h[:], scale=0.5,
                )
                # partition-shift stack-1 half to base 0 (DMA), then add
                xs = xbpool.tile([C, CW], F32, name="xs")
                nc.scalar.dma_start(xs[:], xb2[C:C2, :])
                nc.vector.tensor_add(
                    outT[:, rc * CW:(rc + 1) * CW],
                    xb2[0:C, :], xs[:],
                )

            # phase B: gather half 1 per output half; the first half's
            # epilogue hides under the second half's matmul stream
            for j in range(NCORES):
                p2_block(1, j, 0, [0, 1], j == NCORES - 1)
            for rc in (0, 1):
                p2_epilogue(rc)
            nc.scalar.dma_start(out_e[:, 0:HW_], outT[:, 0:HW_])
            for j in range(NCORES):
                p2_block(1, j, 1, [2, 3], j == NCORES - 1)
            for rc in (2, 3):
                p2_epilogue(rc)
            nc.scalar.dma_start(out_e[:, HW_:R], outT[:, HW_:R])

    nc.compile()
    return nc


def kernel(**inputs):
    x = np.ascontiguousarray(np.asarray(inputs["x"], dtype=np.float32))
    fltr = np.ascontiguousarray(np.asarray(inputs["fltr"], dtype=np.float32))

    def cat(a, b, axis=1):
        return np.ascontiguousarray(
            np.concatenate(
                [np.asarray(a, np.float32), np.asarray(b, np.float32)],
                axis=axis,
            )
        )

    f8 = ml_dtypes.float8_e3m4
    bf = ml_dtypes.bfloat16
    w1i0 = np.ascontiguousarray(
        (cat(inputs["k0i0_w1"], inputs["k1i0_w1"]) / FSCALE).astype(bf))
    w1i1f = np.zeros((C2, C2), dtype=np.float32)
    w1i1f[0:C, 0:C] = np.asarray(inputs["k0i1_w1"], np.float32)
    w1i1f[C:C2, C:C2] = np.asarray(inputs["k1i1_w1"], np.float32)
    w1i1 = np.ascontiguousarray((w1i1f / FSCALE).astype(bf))
    w2i0 = cat(inputs["k0i0_w2"], inputs["k1i0_w2"])
    w2i1 = cat(inputs["k0i1_w2"], inputs["k1i1_w2"])
    bi0 = cat(inputs["k0i0_b"], inputs["k1i0_b"], axis=0)[:, None]
    bi1h = 0.5 * cat(inputs["k0i1_b"], inputs["k1i1_b"], axis=0)[:, None]
    bi1h = np.ascontiguousarray(bi1h)
    xT = np.ascontiguousarray(x.T.astype(bf))
    # fp8 E3M4 fltr at rest: transpose per core, scale by 2^8 (descale is
    # folded into w1i0/w1i1 above; values land in [-10.9, 10.9] < 15.5 max)
    fltrs = (fltr * np.float32(FSCALE)).astype(f8)

    if "nc" not in _CACHE:
        _CACHE["nc"] = _build()
    nc = _CACHE["nc"]

    in_maps = []
    for m in range(NCORES):
        rows = slice(m * R, (m + 1) * R)
        in_maps.append({
            "fltrt0": np.ascontiguousarray(fltrs[m * R:m * R + HW_, :].T),
            "fltrt1": np.ascontiguousarray(fltrs[m * R + HW_:(m + 1) * R, :].T),
            "xt": xT,
            "xtm": np.ascontiguousarray(x[rows, :].T),
            "w1i0": w1i0, "w1i1": w1i1, "w2i0": w2i0, "w2i1": w2i1,
            "bi0": bi0, "bi1h": bi1h,
        })

    import os
    import time
    trace = os.environ.get("ARMA_TRACE") == "1"
    last_exc = None
    for attempt in range(3):
        try:
            res = run_bass_kernel_spmd(
                nc, in_maps, core_ids=list(range(NCORES)), trace=trace,
            )
            break
        except Exception as e:  # transient NRT device errors: retry
            last_exc = e
            time.sleep(5.0)
    else:
        raise last_exc
    _CACHE["last_results"] = res
    out = np.concatenate(
        [np.asarray(res.results[m]["out"]).T for m in range(NCORES)], axis=0
    )
    return out
